# revision 7
# baseline (speedup 1.0000x reference)
"""CMRF kernel for 8 trn2 NeuronCores.

Math (reference):
  sigx = sigmoid(lam*x); disc = (sign(x-.5)+1)/2
  expterm[f,r,s] = exp(sum_v A[f,r,v]*sigx[s,vl[f,v]] + B[f,r])
  Ph[f,s] = sum_r expterm ;  Phat = prod_f Ph
  P       = prod_f sum_r exp(sum_v A*disc_g + B)
  contrib[s,f,v] = (sum_r A[f,r,v] expterm[f,r,s]) / Ph[f,s]
  Dx[s, vl[f,v]] += contrib ; Dx *= dx ; w = (Phat/P)/sum(Phat/P)

Sharding: factor axis F split across 8 cores (2048 factors each).

Device program per core (identical SPMD, different data):
  - host gathers x columns per vlabel -> sgx (bf16, [4096,512]) laid out so
    each z-tile (32 factors x 4 r) reads a K=64 window [32 rows v0 | 32 rows v1]
  - tanh(lam/2 * x) on ACT; sigmoid folded into weights:
      sig = .5 + .5*tanh  =>  z+B = sum_v .5*A_v*tanh_v + B', B'=B+.5*(A0+A1)
  - z via PE matmuls with block-diagonal lhsT (host-built), exp on ACT
    (bias-free: e^{B'} folded into the selector weights)
  - Ph/N0/N1 via PE selector matmuls (col-tiled, M=32)
  - ln(Ph) on ACT; 1/Ph via DVE reciprocal_approx_fast; contrib = N*inv on DVE
  - disc side: host sends (x>.5) masks as uint8 (cast-DMA to f32); log P is
    bilinear in the masks => pure M=1 matmuls with host log-q coefficients
  - one PSUM row accumulates sum_f[ln Ph] - sum_f[ln P_f] (order-free adds)
Host combine: softmax(sum of acc rows) -> w_s ; sorted reduceat scatter-add of
contrib -> Dx ; multiply by dx. Exact handling of x==0.5 via host patching.
"""

import numpy as np
import ml_dtypes

S, D, F, R, V = 512, 4096, 16384, 4, 2
NCORES = 8
FC = F // NCORES          # 2048 factors per core
NZT = FC // 32            # 64 z-tiles (32 factors x 4 r = 128 M)
NST = 8                   # super-tiles: 512 sgx rows each (8 z-tiles, 2 pairs)
NPAIR = FC // 128         # 16 pairs (128 factors)
NG2 = FC // 64            # 32 disc groups of 64 factors

_NC_CACHE = {}
LAST_EXEC_NS = None
TRACE = False

bf16 = ml_dtypes.bfloat16


def _build_nc(lam_half: float):
    import concourse.bacc as bacc
    import concourse.tile as tile
    from concourse import mybir
    from contextlib import ExitStack

    dt = mybir.dt
    AF = mybir.ActivationFunctionType
    OP = mybir.AluOpType

    nc = bacc.Bacc("TRN2", target_bir_lowering=False, debug=False)

    sgx = nc.dram_tensor("sgx", [2 * FC, S], dt.bfloat16, kind="ExternalInput")
    wz = nc.dram_tensor("wz", [128, (NZT // 2) * 128], dt.float32, kind="ExternalInput")
    wph = nc.dram_tensor("wph", [128, NZT * 32], dt.float32, kind="ExternalInput")
    wn0 = nc.dram_tensor("wn0", [128, NZT * 32], dt.float32, kind="ExternalInput")
    wn1 = nc.dram_tensor("wn1", [128, NZT * 32], dt.float32, kind="ExternalInput")
    wd1 = nc.dram_tensor("wd1", [128, NG2], dt.float32, kind="ExternalInput")
    wd2 = nc.dram_tensor("wd2", [64, NG2], dt.float32, kind="ExternalInput")
    dsc = nc.dram_tensor("dsc", [NG2 * 128, S], dt.uint8, kind="ExternalInput")
    d01 = nc.dram_tensor("d01", [NG2 * 64, S], dt.uint8, kind="ExternalInput")
    contrib = nc.dram_tensor("contrib", [2 * FC, S], dt.float32, kind="ExternalOutput")
    accrow = nc.dram_tensor("accrow", [1, S], dt.float32, kind="ExternalOutput")

    with tile.TileContext(nc) as tc, ExitStack() as ctx:
        singles = ctx.enter_context(tc.tile_pool(name="singles", bufs=1))
        sgxp = ctx.enter_context(tc.tile_pool(name="sgxp", bufs=2))
        sgp = ctx.enter_context(tc.tile_pool(name="sgp", bufs=2))
        ep = ctx.enter_context(tc.tile_pool(name="ep", bufs=2))
        lp = ctx.enter_context(tc.tile_pool(name="lp", bufs=2))
        ivp = ctx.enter_context(tc.tile_pool(name="ivp", bufs=2))
        cp = ctx.enter_context(tc.tile_pool(name="cp", bufs=4))
        dp = ctx.enter_context(tc.tile_pool(name="dp", bufs=2))
        d01p = ctx.enter_context(tc.tile_pool(name="d01p", bufs=2))
        zp = ctx.enter_context(tc.tile_pool(name="zp", bufs=1, space="PSUM"))
        php = ctx.enter_context(tc.tile_pool(name="php", bufs=1, space="PSUM"))
        n0p = ctx.enter_context(tc.tile_pool(name="n0p", bufs=1, space="PSUM"))
        n1p = ctx.enter_context(tc.tile_pool(name="n1p", bufs=1, space="PSUM"))
        accp = ctx.enter_context(tc.tile_pool(name="accp", bufs=1, space="PSUM"))

        wzall = singles.tile([128, (NZT // 2) * 128], dt.float32)
        nc.sync.dma_start(out=wzall[:, :], in_=wz[:, :])
        wphall = singles.tile([128, NZT * 32], dt.float32)
        nc.sync.dma_start(out=wphall[:, :], in_=wph[:, :])
        wn0all = singles.tile([128, NZT * 32], dt.float32)
        nc.sync.dma_start(out=wn0all[:, :], in_=wn0[:, :])
        wn1all = singles.tile([128, NZT * 32], dt.float32)
        nc.sync.dma_start(out=wn1all[:, :], in_=wn1[:, :])
        wd1all = singles.tile([128, NG2], dt.float32)
        nc.sync.dma_start(out=wd1all[:, :], in_=wd1[:, :])
        wd2all = singles.tile([64, NG2], dt.float32)
        nc.sync.dma_start(out=wd2all[:, :], in_=wd2[:, :])

        ones = singles.tile([128, 1], dt.float32)
        nc.vector.memset(ones[:, :], 1.0)
        zrow = singles.tile([1, S], dt.float32)
        nc.vector.memset(zrow[:, :], 0.0)

        acc = accp.tile([1, S], dt.float32)
        # open the accumulation group with a zeroing matmul (1.0 x zeros);
        # all later adds are commutative, so scheduler order doesn't matter
        nc.tensor.matmul(acc[:, :], lhsT=ones[0:1, 0:1], rhs=zrow[:, :],
                         start=True, stop=False, skip_group_check=True)

        def accmm(lhsT, rhs):
            nc.tensor.matmul(acc[:, :], lhsT=lhsT, rhs=rhs,
                             start=False, stop=False, skip_group_check=True)

        for t in range(NST):
            sgxt = sgxp.tile([128, 4, S], dt.bfloat16)
            nc.sync.dma_start(
                out=sgxt[:, :, :],
                in_=sgx[t * 512:(t + 1) * 512, :].rearrange("(j p) s -> p j s", p=128),
            )
            sgt = sgp.tile([128, 4, S], dt.float32)
            nc.scalar.activation(sgt[:, :, :], sgxt[:, :, :], AF.Tanh, scale=lam_half)

            for q in range(2):
                p_idx = t * 2 + q
                zt0 = t * 8 + q * 4
                zps = zp.tile([128, 4, S], dt.float32)
                for i in range(4):
                    zt = zt0 + i
                    nc.tensor.matmul(
                        zps[:, i, :],
                        lhsT=wzall[(zt % 2) * 64:(zt % 2) * 64 + 64,
                                   (zt // 2) * 128:(zt // 2) * 128 + 128],
                        rhs=sgt[(zt % 2) * 64:(zt % 2) * 64 + 64, (zt % 8) // 2, :],
                        start=True, stop=True,
                    )
                et = ep.tile([128, 4, S], dt.float32)
                nc.scalar.activation(et[:, :, :], zps[:, :, :], AF.Exp)

                pht = php.tile([128, S], dt.float32)
                n0t = n0p.tile([128, S], dt.float32)
                n1t = n1p.tile([128, S], dt.float32)
                for i in range(4):
                    zt = zt0 + i
                    sl = slice(zt * 32, (zt + 1) * 32)
                    nc.tensor.matmul(pht[i * 32:(i + 1) * 32, :],
                                     lhsT=wphall[:, sl], rhs=et[:, i, :],
                                     start=True, stop=True, tile_position=(0, i * 32))
                    nc.tensor.matmul(n0t[i * 32:(i + 1) * 32, :],
                                     lhsT=wn0all[:, sl], rhs=et[:, i, :],
                                     start=True, stop=True, tile_position=(0, i * 32))
                    nc.tensor.matmul(n1t[i * 32:(i + 1) * 32, :],
                                     lhsT=wn1all[:, sl], rhs=et[:, i, :],
                                     start=True, stop=True, tile_position=(0, i * 32))

                lpt = lp.tile([128, S], dt.float32)
                nc.scalar.activation(lpt[:, :], pht[:, :], AF.Ln)
                ivt = ivp.tile([128, S], dt.float32)
                nc.vector.reciprocal_approx_fast(out=ivt[:, :], in_=pht[:, :])
                c0 = cp.tile([128, S], dt.float32)
                c1 = cp.tile([128, S], dt.float32)
                nc.vector.tensor_tensor(out=c0[:, :], in0=n0t[:, :], in1=ivt[:, :],
                                        op=OP.mult)
                nc.vector.tensor_tensor(out=c1[:, :], in0=n1t[:, :], in1=ivt[:, :],
                                        op=OP.mult)
                nc.sync.dma_start(out=contrib[p_idx * 128:(p_idx + 1) * 128, :],
                                  in_=c0[:, :])
                nc.sync.dma_start(out=contrib[FC + p_idx * 128:FC + (p_idx + 1) * 128, :],
                                  in_=c1[:, :])
                accmm(ones[:, 0:1], lpt[:, :])

            dct = dp.tile([128, 4, S], dt.float32)
            nc.gpsimd.dma_start(
                out=dct[:, :, :],
                in_=dsc[t * 512:(t + 1) * 512, :].rearrange("(j p) s -> p j s", p=128),
            )
            d01t = d01p.tile([64, 4, S], dt.float32)
            nc.gpsimd.dma_start(
                out=d01t[:, :, :],
                in_=d01[t * 256:(t + 1) * 256, :].rearrange("(j p) s -> p j s", p=64),
            )
            for j in range(4):
                g2 = t * 4 + j
                accmm(wd1all[:, g2:g2 + 1], dct[:, j, :])
                accmm(wd2all[:, g2:g2 + 1], d01t[:, j, :])

        accs = singles.tile([1, S], dt.float32)
        nc.vector.tensor_copy(out=accs[:, :], in_=acc[:, :])
        nc.sync.dma_start(out=accrow[:, :], in_=accs[:, :])

    nc.compile()
    return nc


def _get_nc(lam_half: float):
    key = round(float(lam_half), 12)
    if key not in _NC_CACHE:
        _NC_CACHE[key] = _build_nc(float(lam_half))
    return _NC_CACHE[key]


def _host_prep(x, lam, A, B, vlabel):
    """Build per-core input maps + metadata for the combine step."""
    in_maps = []
    metas = []
    for c in range(NCORES):
        vl = vlabel[c * FC:(c + 1) * FC, :]          # [2048, 2]
        Ac = A[c * FC:(c + 1) * FC]                  # [2048, 4, 2]
        Bc = B[c * FC:(c + 1) * FC]                  # [2048, 4]

        # --- gathered x, z-tile interleave: rows zt*64 + [32 v0 | 32 v1]
        vr = vl.reshape(NZT, 32, 2)
        colmat = np.concatenate([vr[:, :, 0], vr[:, :, 1]], axis=1).reshape(-1)
        xg = x[:, colmat].T                          # [4096, 512] f32
        sgx_np = np.ascontiguousarray(xg.astype(bf16))

        # --- disc masks, g2 interleave: rows g2*128 + [64 v0 | 64 v1]
        d0 = (x[:, vl[:, 0]] > 0.5)                  # [512, 2048]
        d1 = (x[:, vl[:, 1]] > 0.5)
        dr0 = d0.T.reshape(NG2, 64, S)
        dr1 = d1.T.reshape(NG2, 64, S)
        dsc_np = np.ascontiguousarray(
            np.concatenate([dr0, dr1], axis=1).reshape(NG2 * 128, S).astype(np.uint8))
        d01_np = np.ascontiguousarray((d0 & d1).T.astype(np.uint8))  # [2048, 512]

        # --- z weights: sig = .5 + .5*tanh  (0.5*A folded); B' = B + .5*sum_v A
        Bp = Bc + 0.5 * (Ac[:, :, 0] + Ac[:, :, 1])
        EB = np.exp(Bp)                               # [2048, 4]
        fl = np.arange(32)
        r_ = np.arange(4)
        blk = np.zeros((NZT, 64, 128), np.float32)
        cols = (fl[:, None] * 4 + r_[None, :])        # [32, 4]
        for v, roff in ((0, 0), (1, 32)):
            av = 0.5 * Ac[:, :, v].reshape(NZT, 32, 4)   # [64, 32, 4]
            blk[:, fl[:, None] + roff, cols] = av
        wz_np = np.ascontiguousarray(
            blk.reshape(NZT // 2, 2, 64, 128).transpose(1, 2, 0, 3)
            .reshape(128, (NZT // 2) * 128).astype(np.float32))

        # --- selector weights with e^{B'} folded in
        def selw(vals):  # vals [2048, 4] per (f, r)
            wb = np.zeros((NZT, 128, 32), np.float32)
            vv = vals.reshape(NZT, 32, 4)
            wb[:, cols, fl[:, None]] = vv
            return np.ascontiguousarray(
                wb.transpose(1, 0, 2).reshape(128, NZT * 32).astype(np.float32))

        wph_np = selw(EB)
        wn0_np = selw(Ac[:, :, 0] * EB)
        wn1_np = selw(Ac[:, :, 1] * EB)

        # --- disc log-q bilinear coefficients (negated: acc = logPh - logP)
        corners0 = np.array([0.0, 1.0, 0.0, 1.0], np.float32)
        corners1 = np.array([0.0, 0.0, 1.0, 1.0], np.float32)
        z4 = (Bc[:, :, None] + Ac[:, :, 0:1] * corners0 + Ac[:, :, 1:2] * corners1)
        L = np.log(np.exp(z4.astype(np.float64)).sum(axis=1))  # [2048, 4]
        L00, L10, L01, L11 = L[:, 0], L[:, 1], L[:, 2], L[:, 3]
        c10 = L10 - L00
        c01 = L01 - L00
        c11 = L11 - L10 - L01 + L00
        wd1_np = np.zeros((128, NG2), np.float32)
        wd1_np[0:64, :] = -c10.reshape(NG2, 64).T
        wd1_np[64:128, :] = -c01.reshape(NG2, 64).T
        wd2_np = np.ascontiguousarray((-c11.reshape(NG2, 64).T).astype(np.float32))

        in_maps.append(dict(
            sgx=sgx_np, wz=wz_np, wph=wph_np, wn0=wn0_np, wn1=wn1_np,
            wd1=np.ascontiguousarray(wd1_np), wd2=wd2_np, dsc=dsc_np, d01=d01_np,
        ))
        metas.append(dict(vl=vl, Ac=Ac, Bc=Bc, L00=L00, c10=c10, c01=c01, c11=c11,
                          d0=d0, d1=d1))
    return in_maps, metas


def _combine(x, lam, vlabel, results, metas):
    """Host unshard: softmax of acc rows -> w ; scatter-add contrib -> Dx."""
    acc = np.zeros(S, np.float64)
    for c in range(NCORES):
        acc += results[c]["accrow"][0].astype(np.float64)

    # exact patch for x == 0.5 at gathered positions (bilinear is only exact
    # for d in {0,1}; reference uses d = 0.5 there)
    half0 = (x[:, vlabel[:, 0]] == 0.5)
    half1 = (x[:, vlabel[:, 1]] == 0.5)
    bad_s = np.nonzero(half0.any(axis=1) | half1.any(axis=1))[0]
    for s in bad_s:
        d0 = (x[s, vlabel[:, 0]] > 0.5) + 0.5 * half0[s]
        d1 = (x[s, vlabel[:, 1]] > 0.5) + 0.5 * half1[s]
        # device used the binary >0.5 masks; replace its bilinear logP with the
        # exact logP at d in {0, .5, 1}. L00 constants cancel in the difference.
        logP_exact = 0.0
        logP_dev = 0.0
        for c in range(NCORES):
            m = metas[c]
            dd0 = d0[c * FC:(c + 1) * FC]
            dd1 = d1[c * FC:(c + 1) * FC]
            zc = (m["Bc"].astype(np.float64)
                  + m["Ac"][:, :, 0] * dd0[:, None] + m["Ac"][:, :, 1] * dd1[:, None])
            logP_exact += np.log(np.exp(zc).sum(axis=1)).sum()
            b0 = m["d0"][s].astype(np.float64)
            b1 = m["d1"][s].astype(np.float64)
            logP_dev += (m["L00"] + m["c10"] * b0 + m["c01"] * b1
                         + m["c11"] * b0 * b1).sum()
        acc[s] += (logP_dev - logP_exact)

    acc -= acc.max()
    w = np.exp(acc)
    w_s = (w / w.sum()).astype(np.float32)

    # ---- Dx scatter-add
    Cs = [results[c]["contrib"] for c in range(NCORES)]      # each [4096, 512]
    Cflat = np.concatenate(Cs, axis=0)                       # [32768, 512]
    tgt = np.concatenate(
        [np.concatenate([metas[c]["vl"][:, 0], metas[c]["vl"][:, 1]])
         for c in range(NCORES)]).astype(np.int64)
    order = np.argsort(tgt, kind="stable")
    su = tgt[order]
    uniq, starts = np.unique(su, return_index=True)
    sums = np.add.reduceat(Cflat[order], starts, axis=0)
    DxT = np.zeros((D, S), np.float32)
    DxT[uniq] = sums

    expx = np.exp(-lam * x)
    dx = lam * expx / (1.0 + expx) ** 2
    Dx = (DxT.T * dx).astype(np.float32)
    return w_s, Dx


def _install_ntff_hook():
    """Provide antenv.axon_hooks (missing in this image) so trace=True works."""
    import sys
    import types
    if "antenv.axon_hooks" in sys.modules:
        return
    try:
        from trn_agent_boot.trn_boot import _ntff_profile_via_ctypes
        raw = _ntff_profile_via_ctypes("/opt/axon/libaxon_pjrt.so")
        if raw is None:
            hook = None
        else:
            import os as _os
            mode = _os.environ.get("KTRACE_DEVS", "as-is")

            def hook(output_dir, device_ids, _raw=raw, _mode=mode):
                if _mode == "all":
                    device_ids = None
                return _raw(output_dir, device_ids)
    except Exception:
        hook = None
    m = types.ModuleType("antenv.axon_hooks")
    m.get_axon_ntff_profile_hook = lambda: hook
    m.set_axon_ntff_profile_hook = lambda h: None
    sys.modules["antenv.axon_hooks"] = m


def kernel(**inputs):
    global LAST_EXEC_NS
    x = np.asarray(inputs["x"], np.float32)
    lam = float(np.asarray(inputs["lamda"]))
    A = np.asarray(inputs["A"], np.float32)
    B = np.asarray(inputs["B"], np.float32)
    vlabel = np.asarray(inputs["vlabel"])

    in_maps, metas = _host_prep(x, lam, A, B, vlabel)
    nc = _get_nc(lam / 2.0)

    if TRACE:
        _install_ntff_hook()
    from concourse.bass_utils import run_bass_kernel_spmd
    res = run_bass_kernel_spmd(nc, in_maps, core_ids=list(range(NCORES)),
                               trace=TRACE)
    LAST_EXEC_NS = res.exec_time_ns
    return _combine(x, lam, vlabel, res.results, metas)
